# revision 3
# baseline (speedup 1.0000x reference)
"""DeFeat distillation loss on 8 Trainium2 NeuronCores (Bass/Tile).

Data-parallel over the batch dim (B=8 -> 1 batch element per core).
Host packs all 5 pyramid levels into ONE [128, 2*21824] f32 array per
feature tensor (per block: [lo-half cols | hi-half cols]), so the kernel
streams uniform large-line DMAs with a single dma_start per tensor per
block. Per 512-column matmul tile:
  psum = W @ feat_s                      [TensorE fp32r]
  d    = (feat_t - b) - psum   (bf16)    [VectorE fused, frees psum fast]
  dd   = d^2                   (bf16)    [ScalarE Square, segment-wide]
  q    = ones^T @ dd           (psum)    [TensorE bf16: column sums]
  qcat[tile]  = q (bf16 staging row)     [ScalarE copy]
The mask depends only on the column, so the masked sum factors:
  s_gt = sum_n m[n] * q[n],  s_tot = sum_n q[n].
The host rasterizes the masks, takes the per-core q vectors and finishes
both dot products in float64, then applies sqrt + weights.

Block schedule: small blocks first (fast pipeline fill), 2048-wide in
steady state, and a tiny 320-col final block (levels 3+4) so the compute
tail after the last DMA is short.
"""

import os
import sys

for _p in ("/opt/trn_rl_repo", os.path.expanduser("~/.axon_site/_ro/trn_rl_repo")):
    if os.path.isdir(_p) and _p not in sys.path:
        sys.path.insert(0, _p)

import numpy as np

WEIGHT_GT = 0.004
WEIGHT_BG = 0.0002
STRIDES = (8, 16, 32, 64, 128)
SIZES = (128, 64, 32, 16, 8)
HWS = tuple(s * s for s in SIZES)          # (16384, 4096, 1024, 256, 64)
B, C, NBOX = 8, 256, 16
N_CORES = 8
TILE_N = 512                               # matmul free-dim tile
N_LEVELS = 5
MASK_LEN = sum(HWS)                        # 21824
MASK_OFF = tuple(sum(HWS[:i]) for i in range(N_LEVELS))
LEVEL_OFF = MASK_OFF                       # packed column space == level concat

# Widths: narrow head blocks start compute early; 2048 steady-state; the
# last block is levels 3+4 (320 cols) for a short tail.
BLOCK_WIDTHS = (256, 256, 512, 1024) + (2048,) * 9 + (1024, 320)
MAX_BW = max(BLOCK_WIDTHS)
N_WT_CHUNKS = N_LEVELS * 4                 # (lvl, oc, kc) 128x128 chunks


def _layout():
    """Per-block segments/tiles in the packed column space + q-tile table."""
    blocks = []
    q_tiles = []                           # (lvl, level_col, n) per 512-tile
    off = 0
    for w_blk in BLOCK_WIDTHS:
        segs = []                          # (lvl, level_col, block_col, w)
        col, bcol = off, 0
        while col < off + w_blk:
            lvl = max(i for i in range(N_LEVELS) if LEVEL_OFF[i] <= col)
            seg_end = min(off + w_blk, LEVEL_OFF[lvl] + HWS[lvl])
            w = seg_end - col
            segs.append((lvl, col - LEVEL_OFF[lvl], bcol, w))
            col += w
            bcol += w
        mm_tiles = []                      # (block_col, n, lvl, q_index)
        for (lvl, lcol, bcol, w) in segs:
            for j in range(0, w, TILE_N):
                n = min(TILE_N, w - j)
                mm_tiles.append((bcol + j, n, lvl, len(q_tiles)))
                q_tiles.append((lvl, lcol + j, n))
        blocks.append(dict(off=off, w_blk=w_blk, segs=segs, mm_tiles=mm_tiles))
        off += w_blk
    assert off == MASK_LEN
    return blocks, q_tiles


BLOCK_LAYOUT, Q_TILES = _layout()
N_QT = len(Q_TILES)                        # 41
# out DMA split: everything up to the last block's tiles goes out early
N_QT_LAST = len(BLOCK_LAYOUT[-1]["mm_tiles"])
QCOL_A = (N_QT - N_QT_LAST) * TILE_N

# packed column permutation: packed col -> col of [S_lo | S_hi] concat
_perm = []
for _off, _w in zip((b["off"] for b in BLOCK_LAYOUT), BLOCK_WIDTHS):
    _perm.extend(range(_off, _off + _w))
    _perm.extend(range(MASK_LEN + _off, MASK_LEN + _off + _w))
PACK_PERM = np.asarray(_perm, np.int64)


def _build_module():
    import concourse.mybir as mybir
    from concourse import bacc
    from concourse.tile import TileContext

    dt = mybir.dt
    nc = bacc.Bacc("TRN2", target_bir_lowering=False, debug=False,
                   num_devices=N_CORES)

    fs_d = nc.dram_tensor("fs", [128, 2 * MASK_LEN], dt.float32,
                          kind="ExternalInput")
    ft_d = nc.dram_tensor("ft", [128, 2 * MASK_LEN], dt.float32,
                          kind="ExternalInput")
    # weight chunk idx ((lvl*2+oc)*2+kc) at columns idx*128
    wt_d = nc.dram_tensor("wt", [128, N_WT_CHUNKS * 128], dt.float32,
                          kind="ExternalInput")
    bias_d = nc.dram_tensor("bias", [128, N_LEVELS * 2], dt.float32,
                            kind="ExternalInput")
    out_q_d = nc.dram_tensor("out_q", [1, N_QT * TILE_N], dt.bfloat16,
                             kind="ExternalOutput")

    f32r = dt.float32r
    SUB = mybir.AluOpType.subtract
    SQUARE = mybir.ActivationFunctionType.Square

    with TileContext(nc) as tc:
        with (
            tc.tile_pool(name="const", bufs=1) as const_pool,
            tc.tile_pool(name="feat", bufs=3) as feat_pool,
            tc.tile_pool(name="work", bufs=3) as work_pool,
            tc.tile_pool(name="ps", bufs=6, space="PSUM") as psum_pool,
            tc.tile_pool(name="qps", bufs=2, space="PSUM") as qpsum_pool,
        ):
            wt = const_pool.tile([128, N_WT_CHUNKS * 128], f32r)
            bias = const_pool.tile([128, N_LEVELS * 2], dt.float32)
            ones_bf = const_pool.tile([128, 1], dt.bfloat16)
            nc.vector.memset(ones_bf[:], 1.0)
            qcat = const_pool.tile([1, N_QT * TILE_N], dt.bfloat16)
            # zero the partial-tile gap columns (narrow tiles) so the final
            # DMA reads initialized memory
            for _qi, (_lvl, _col, _n) in enumerate(Q_TILES):
                if _n < TILE_N:
                    nc.vector.memset(
                        qcat[:, _qi * TILE_N + _n:(_qi + 1) * TILE_N], 0.0)

            def q_phase(mm_tiles, dd0, dd1):
                # column sums over all 256 channels: q = ones^T @ [dd0;dd1]
                for (bcol, n, lvl, qi) in mm_tiles:
                    qps = qpsum_pool.tile([1, TILE_N], dt.float32, tag="qps")
                    nc.tensor.matmul(qps[:, :n], ones_bf[:],
                                     dd0[:, bcol:bcol + n],
                                     start=True, stop=False)
                    nc.tensor.matmul(qps[:, :n], ones_bf[:],
                                     dd1[:, bcol:bcol + n],
                                     start=False, stop=True)
                    nc.scalar.copy(qcat[:, qi * TILE_N:qi * TILE_N + n],
                                   qps[:, :n])

            pending = None
            for bi, blk in enumerate(BLOCK_LAYOUT):
                w = blk["w_blk"]
                off2 = 2 * blk["off"]
                s_t = feat_pool.tile([128, 2 * MAX_BW], f32r, tag="s")
                t_t = feat_pool.tile([128, 2 * MAX_BW], dt.float32, tag="t")
                nc.sync.dma_start(out=s_t[:, 0:2 * w],
                                  in_=fs_d[:, off2:off2 + 2 * w].bitcast(f32r))
                if bi == 0:
                    # level-0 weights (first 4 chunks) unblock block 0
                    nc.sync.dma_start(out=wt[:, 0:512],
                                      in_=wt_d[:, 0:512].bitcast(f32r))
                nc.sync.dma_start(out=t_t[:, 0:2 * w],
                                  in_=ft_d[:, off2:off2 + 2 * w])
                if bi == 0:
                    nc.sync.dma_start(out=bias[:], in_=bias_d[:])
                elif bi == 1:
                    nc.sync.dma_start(
                        out=wt[:, 512:N_WT_CHUNKS * 128],
                        in_=wt_d[:, 512:N_WT_CHUNKS * 128].bitcast(f32r))

                dd_ocs = []
                for oc in range(2):
                    d_blk = work_pool.tile([128, MAX_BW], dt.bfloat16, tag="d")
                    for (bcol, n, lvl, qi) in blk["mm_tiles"]:
                        widx = (lvl * 2 + oc) * 2
                        ps = psum_pool.tile([128, TILE_N], dt.float32,
                                            tag="ps")
                        nc.tensor.matmul(
                            ps[:, :n],
                            wt[:, widx * 128:(widx + 1) * 128],
                            s_t[:, bcol:bcol + n],
                            start=True, stop=False)
                        nc.tensor.matmul(
                            ps[:, :n],
                            wt[:, (widx + 1) * 128:(widx + 2) * 128],
                            s_t[:, w + bcol:w + bcol + n],
                            start=False, stop=True)
                        # d = (t - b) - psum; frees the psum bank quickly
                        nc.vector.scalar_tensor_tensor(
                            d_blk[:, bcol:bcol + n],
                            t_t[:, oc * w + bcol:oc * w + bcol + n],
                            bias[:, lvl * 2 + oc:lvl * 2 + oc + 1],
                            ps[:, :n],
                            op0=SUB, op1=SUB)
                    dd_blk = work_pool.tile([128, MAX_BW], dt.bfloat16,
                                            tag=f"dd{oc}")
                    for (lvl, lcol, bcol, sw) in blk["segs"]:
                        nc.scalar.activation(
                            dd_blk[:, bcol:bcol + sw],
                            d_blk[:, bcol:bcol + sw], SQUARE)
                    dd_ocs.append(dd_blk)

                # software-pipelined: emit the PREVIOUS block's q phase so
                # the in-order PE stream never waits on this block's squares
                if pending is not None:
                    q_phase(*pending)
                if bi == len(BLOCK_LAYOUT) - 1:
                    # everything but the last block's q tiles is final now
                    nc.sync.dma_start(out=out_q_d[:, 0:QCOL_A],
                                      in_=qcat[:, 0:QCOL_A])
                pending = (blk["mm_tiles"], dd_ocs[0], dd_ocs[1])

            q_phase(*pending)
            nc.sync.dma_start(out=out_q_d[:, QCOL_A:],
                              in_=qcat[:, QCOL_A:])

    nc.compile()
    return nc


def _rasterize_masks(gt_bboxes):
    """Host-side mask rasterization, mirroring reference.gt_mask in fp32.

    Returns [B, MASK_LEN] float32 (per-level masks concatenated)."""
    out = np.zeros((B, MASK_LEN), np.float32)
    for lvl in range(N_LEVELS):
        h = w = SIZES[lvl]
        stride = np.float32(STRIDES[lvl])
        off = MASK_OFF[lvl]
        q = np.floor(gt_bboxes.astype(np.float32) / stride).astype(np.int32)
        lx = np.minimum(q[..., 0], w - 1)
        ly = np.minimum(q[..., 1], h - 1)
        rx = np.minimum(q[..., 2], w - 1)
        ry = np.minimum(q[..., 3], h - 1)
        for b in range(B):
            m = np.zeros((h, w), bool)
            for i in range(gt_bboxes.shape[1]):
                if lx[b, i] == rx[b, i] or ly[b, i] == ry[b, i]:
                    m[ly[b, i], lx[b, i]] = True
                else:
                    m[ly[b, i]:ry[b, i], lx[b, i]:rx[b, i]] = True
            out[b, off:off + h * w] = m.reshape(-1).astype(np.float32)
    return out


_NC_CACHE = None


def _get_nc():
    global _NC_CACHE
    if _NC_CACHE is None:
        _NC_CACHE = _build_module()
    return _NC_CACHE


def _run(in_maps, trace=False, trace_cores=None):
    from concourse.bass_utils import run_bass_kernel_spmd

    kwargs = {}
    if trace:
        kwargs.update(trace=True, trace_cores=trace_cores or [0])
    return run_bass_kernel_spmd(_get_nc(), in_maps, core_ids=list(range(N_CORES)),
                                **kwargs)


def _pack_const(inputs):
    """Pack replicated weights/bias: chunk ((lvl*2+oc)*2+kc) at idx*128
    holds w_lvl[oc*128+o_local, kc*128+c_local] transposed."""
    wt_packed = np.zeros((128, N_WT_CHUNKS * 128), np.float32)
    bias_packed = np.zeros((128, N_LEVELS * 2), np.float32)
    for lvl in range(N_LEVELS):
        w = np.asarray(inputs[f"adapt_w{lvl}"], np.float32)
        bvec = np.asarray(inputs[f"adapt_b{lvl}"], np.float32)
        for oc in range(2):
            bias_packed[:, lvl * 2 + oc] = bvec[oc * 128:(oc + 1) * 128]
            for kc in range(2):
                idx = (lvl * 2 + oc) * 2 + kc
                blk = w[oc * 128:(oc + 1) * 128, kc * 128:(kc + 1) * 128]
                wt_packed[:, idx * 128:(idx + 1) * 128] = blk.T
    return wt_packed, bias_packed


def _pack_feat(full, b):
    """[C,*,*] levels of batch item b -> packed [128, 2*MASK_LEN]."""
    cat = np.concatenate(
        [np.asarray(full[l][b], np.float32).reshape(C, HWS[l])
         for l in range(N_LEVELS)], axis=1)          # [256, 21824]
    lohi = np.concatenate([cat[0:128], cat[128:256]], axis=1)  # [128, 2*21824]
    return np.ascontiguousarray(lohi[:, PACK_PERM])


def kernel(_trace=False, _return_results=False, **inputs):
    gt_bboxes = np.asarray(inputs["gt_bboxes"], np.float32)
    masks = _rasterize_masks(gt_bboxes)
    wt_packed, bias_packed = _pack_const(inputs)

    fs_full = [inputs[f"feat_s{l}"] for l in range(N_LEVELS)]
    ft_full = [inputs[f"feat_t{l}"] for l in range(N_LEVELS)]
    in_maps = []
    for b in range(N_CORES):
        in_maps.append({
            "wt": wt_packed, "bias": bias_packed,
            "fs": _pack_feat(fs_full, b),
            "ft": _pack_feat(ft_full, b),
        })

    res = _run(in_maps, trace=_trace)

    s_tot = np.zeros(N_LEVELS, np.float64)
    s_gt = np.zeros(N_LEVELS, np.float64)
    for c in range(N_CORES):
        q = res.results[c]["out_q"].astype(np.float64).reshape(-1)
        for qi, (lvl, col, n) in enumerate(Q_TILES):
            qv = q[qi * TILE_N:qi * TILE_N + n]
            mv = masks[c, MASK_OFF[lvl] + col:MASK_OFF[lvl] + col + n].astype(np.float64)
            s_tot[lvl] += qv.sum()
            s_gt[lvl] += (qv * mv).sum()

    loss = np.float64(0.0)
    for lvl in range(N_LEVELS):
        s_bg = s_tot[lvl] - s_gt[lvl]
        loss += WEIGHT_GT * np.sqrt(s_gt[lvl] + 1e-8) + \
            WEIGHT_BG * np.sqrt(s_bg + 1e-8)

    out = np.array(loss, dtype=np.float32)
    if _return_results:
        return out, res
    return out


# revision 5
# speedup vs baseline: 1.1438x; 1.1438x over previous
"""DeFeat distillation loss on 8 Trainium2 NeuronCores (Bass/Tile).

Data-parallel over the batch dim (B=8 -> 1 batch element per core).
Host packs all 5 pyramid levels into ONE [128, 2*21824] f32 array per
feature tensor (per block: [lo-half cols | hi-half cols]), so the kernel
streams uniform large-line DMAs with a single dma_start per tensor per
block. Per 512-column matmul tile:
  psum = W @ feat_s                      [TensorE fp32r]
  d    = (feat_t - b) - psum   (bf16)    [VectorE fused, frees psum fast]
  dd   = d^2                   (bf16)    [ScalarE Square, segment-wide]
  q    = ones^T @ dd           (psum)    [TensorE bf16: column sums]
  qcat[tile]  = q (bf16 staging row)     [ScalarE copy]
The mask depends only on the column, so the masked sum factors:
  s_gt = sum_n m[n] * q[n],  s_tot = sum_n q[n].
The host rasterizes the masks, takes the per-core q vectors and finishes
both dot products in float64, then applies sqrt + weights.

Block schedule: small blocks first (fast pipeline fill), 2048-wide in
steady state, and a tiny 320-col final block (levels 3+4) so the compute
tail after the last DMA is short.
"""

import os
import sys

for _p in ("/opt/trn_rl_repo", os.path.expanduser("~/.axon_site/_ro/trn_rl_repo")):
    if os.path.isdir(_p) and _p not in sys.path:
        sys.path.insert(0, _p)

import numpy as np

WEIGHT_GT = 0.004
WEIGHT_BG = 0.0002
STRIDES = (8, 16, 32, 64, 128)
SIZES = (128, 64, 32, 16, 8)
HWS = tuple(s * s for s in SIZES)          # (16384, 4096, 1024, 256, 64)
B, C, NBOX = 8, 256, 16
N_CORES = 8
TILE_N = 512                               # matmul free-dim tile
N_LEVELS = 5
MASK_LEN = sum(HWS)                        # 21824
MASK_OFF = tuple(sum(HWS[:i]) for i in range(N_LEVELS))
LEVEL_OFF = MASK_OFF                       # packed column space == level concat

# Widths: narrow head blocks start compute early; 2048 steady-state; the
# last block is levels 3+4 (320 cols) for a short tail.
BLOCK_WIDTHS = (512, 1536) + (2048,) * 9 + (1024, 320)
MAX_BW = max(BLOCK_WIDTHS)
N_WT_CHUNKS = N_LEVELS * 4                 # (lvl, oc, kc) 128x128 chunks


def _layout():
    """Per-block segments/tiles in the packed column space + q-tile table."""
    blocks = []
    q_tiles = []                           # (lvl, level_col, n) per 512-tile
    off = 0
    for w_blk in BLOCK_WIDTHS:
        segs = []                          # (lvl, level_col, block_col, w)
        col, bcol = off, 0
        while col < off + w_blk:
            lvl = max(i for i in range(N_LEVELS) if LEVEL_OFF[i] <= col)
            seg_end = min(off + w_blk, LEVEL_OFF[lvl] + HWS[lvl])
            w = seg_end - col
            segs.append((lvl, col - LEVEL_OFF[lvl], bcol, w))
            col += w
            bcol += w
        mm_tiles = []                      # (block_col, n, lvl, q_index)
        for (lvl, lcol, bcol, w) in segs:
            for j in range(0, w, TILE_N):
                n = min(TILE_N, w - j)
                mm_tiles.append((bcol + j, n, lvl, len(q_tiles)))
                q_tiles.append((lvl, lcol + j, n))
        blocks.append(dict(off=off, w_blk=w_blk, segs=segs, mm_tiles=mm_tiles))
        off += w_blk
    assert off == MASK_LEN
    return blocks, q_tiles


BLOCK_LAYOUT, Q_TILES = _layout()
N_QT = len(Q_TILES)                        # 41
# out DMA split: everything up to the last block's tiles goes out early
N_QT_LAST = len(BLOCK_LAYOUT[-1]["mm_tiles"])
QCOL_A = (N_QT - N_QT_LAST) * TILE_N

# packed column permutation: packed col -> col of [S_lo | S_hi] concat
_perm = []
for _off, _w in zip((b["off"] for b in BLOCK_LAYOUT), BLOCK_WIDTHS):
    _perm.extend(range(_off, _off + _w))
    _perm.extend(range(MASK_LEN + _off, MASK_LEN + _off + _w))
PACK_PERM = np.asarray(_perm, np.int64)


def _build_module():
    import concourse.mybir as mybir
    from concourse import bacc
    from concourse.tile import TileContext

    dt = mybir.dt
    nc = bacc.Bacc("TRN2", target_bir_lowering=False, debug=False,
                   num_devices=N_CORES)

    fs_d = nc.dram_tensor("fs", [128, 2 * MASK_LEN], dt.float32,
                          kind="ExternalInput")
    ft_d = nc.dram_tensor("ft", [128, 2 * MASK_LEN], dt.float32,
                          kind="ExternalInput")
    # weight chunk idx ((lvl*2+oc)*2+kc) at columns idx*128
    wt_d = nc.dram_tensor("wt", [128, N_WT_CHUNKS * 128], dt.float32,
                          kind="ExternalInput")
    bias_d = nc.dram_tensor("bias", [128, N_LEVELS * 2], dt.float32,
                            kind="ExternalInput")
    out_q_d = nc.dram_tensor("out_q", [1, N_QT * TILE_N], dt.bfloat16,
                             kind="ExternalOutput")

    f32r = dt.float32r
    SUB = mybir.AluOpType.subtract
    SQUARE = mybir.ActivationFunctionType.Square

    with TileContext(nc) as tc:
        with (
            tc.tile_pool(name="const", bufs=1) as const_pool,
            tc.tile_pool(name="feat", bufs=3) as feat_pool,
            tc.tile_pool(name="work", bufs=3) as work_pool,
            tc.tile_pool(name="ps", bufs=6, space="PSUM") as psum_pool,
            tc.tile_pool(name="qps", bufs=2, space="PSUM") as qpsum_pool,
        ):
            wt = const_pool.tile([128, N_WT_CHUNKS * 128], f32r)
            bias = const_pool.tile([128, N_LEVELS * 2], dt.float32)
            ones_bf = const_pool.tile([128, 1], dt.bfloat16)
            nc.vector.memset(ones_bf[:], 1.0)
            qcat = const_pool.tile([1, N_QT * TILE_N], dt.bfloat16)
            # zero the partial-tile gap columns (narrow tiles) so the final
            # DMA reads initialized memory
            for _qi, (_lvl, _col, _n) in enumerate(Q_TILES):
                if _n < TILE_N:
                    nc.vector.memset(
                        qcat[:, _qi * TILE_N + _n:(_qi + 1) * TILE_N], 0.0)

            def q_phase(mm_tiles, dd0, dd1):
                # column sums over all 256 channels: q = ones^T @ [dd0;dd1]
                for (bcol, n, lvl, qi) in mm_tiles:
                    qps = qpsum_pool.tile([1, TILE_N], dt.float32, tag="qps")
                    nc.tensor.matmul(qps[:, :n], ones_bf[:],
                                     dd0[:, bcol:bcol + n],
                                     start=True, stop=False)
                    nc.tensor.matmul(qps[:, :n], ones_bf[:],
                                     dd1[:, bcol:bcol + n],
                                     start=False, stop=True)
                    nc.scalar.copy(qcat[:, qi * TILE_N:qi * TILE_N + n],
                                   qps[:, :n])

            pending = None
            for bi, blk in enumerate(BLOCK_LAYOUT):
                w = blk["w_blk"]
                off2 = 2 * blk["off"]
                s_t = feat_pool.tile([128, 2 * MAX_BW], f32r, tag="s")
                t_t = feat_pool.tile([128, 2 * MAX_BW], dt.float32, tag="t")
                # fine-grained lo/hi DMAs keep the matmul->STT pipeline
                # tight (psum banks drain early; PE never bulk-stalls)
                nc.sync.dma_start(out=s_t[:, 0:w],
                                  in_=fs_d[:, off2:off2 + w].bitcast(f32r))
                if bi == 0:
                    # level-0 weights (first 4 chunks) unblock block 0
                    nc.sync.dma_start(out=wt[:, 0:512],
                                      in_=wt_d[:, 0:512].bitcast(f32r))
                nc.sync.dma_start(out=s_t[:, w:2 * w],
                                  in_=fs_d[:, off2 + w:off2 + 2 * w].bitcast(f32r))
                nc.sync.dma_start(out=t_t[:, 0:w], in_=ft_d[:, off2:off2 + w])
                nc.sync.dma_start(out=t_t[:, w:2 * w],
                                  in_=ft_d[:, off2 + w:off2 + 2 * w])
                if bi == 1:
                    nc.sync.dma_start(
                        out=wt[:, 512:N_WT_CHUNKS * 128],
                        in_=wt_d[:, 512:N_WT_CHUNKS * 128].bitcast(f32r))
                elif bi == 2:
                    nc.sync.dma_start(out=bias[:], in_=bias_d[:])

                dd_ocs = []
                for oc in range(2):
                    d_blk = work_pool.tile([128, MAX_BW], dt.bfloat16, tag="d")
                    for (bcol, n, lvl, qi) in blk["mm_tiles"]:
                        widx = (lvl * 2 + oc) * 2
                        ps = psum_pool.tile([128, TILE_N], dt.float32,
                                            tag="ps")
                        nc.tensor.matmul(
                            ps[:, :n],
                            wt[:, widx * 128:(widx + 1) * 128],
                            s_t[:, bcol:bcol + n],
                            start=True, stop=False)
                        nc.tensor.matmul(
                            ps[:, :n],
                            wt[:, (widx + 1) * 128:(widx + 2) * 128],
                            s_t[:, w + bcol:w + bcol + n],
                            start=False, stop=True)
                        # d = (t - b) - psum; frees the psum bank quickly
                        nc.vector.scalar_tensor_tensor(
                            d_blk[:, bcol:bcol + n],
                            t_t[:, oc * w + bcol:oc * w + bcol + n],
                            bias[:, lvl * 2 + oc:lvl * 2 + oc + 1],
                            ps[:, :n],
                            op0=SUB, op1=SUB)
                    dd_blk = work_pool.tile([128, MAX_BW], dt.bfloat16,
                                            tag=f"dd{oc}")
                    for (lvl, lcol, bcol, sw) in blk["segs"]:
                        nc.scalar.activation(
                            dd_blk[:, bcol:bcol + sw],
                            d_blk[:, bcol:bcol + sw], SQUARE)
                    dd_ocs.append(dd_blk)

                # software-pipelined: emit the PREVIOUS block's q phase so
                # the in-order PE stream never waits on this block's squares
                if pending is not None:
                    q_phase(*pending)
                if bi == len(BLOCK_LAYOUT) - 1:
                    # everything but the last block's q tiles is final now
                    nc.sync.dma_start(out=out_q_d[:, 0:QCOL_A],
                                      in_=qcat[:, 0:QCOL_A])
                pending = (blk["mm_tiles"], dd_ocs[0], dd_ocs[1])

            q_phase(*pending)
            nc.sync.dma_start(out=out_q_d[:, QCOL_A:],
                              in_=qcat[:, QCOL_A:])

    nc.compile()
    return nc


def _rasterize_masks(gt_bboxes):
    """Host-side mask rasterization, mirroring reference.gt_mask in fp32.

    Returns [B, MASK_LEN] float32 (per-level masks concatenated)."""
    out = np.zeros((B, MASK_LEN), np.float32)
    for lvl in range(N_LEVELS):
        h = w = SIZES[lvl]
        stride = np.float32(STRIDES[lvl])
        off = MASK_OFF[lvl]
        q = np.floor(gt_bboxes.astype(np.float32) / stride).astype(np.int32)
        lx = np.minimum(q[..., 0], w - 1)
        ly = np.minimum(q[..., 1], h - 1)
        rx = np.minimum(q[..., 2], w - 1)
        ry = np.minimum(q[..., 3], h - 1)
        for b in range(B):
            m = np.zeros((h, w), bool)
            for i in range(gt_bboxes.shape[1]):
                if lx[b, i] == rx[b, i] or ly[b, i] == ry[b, i]:
                    m[ly[b, i], lx[b, i]] = True
                else:
                    m[ly[b, i]:ry[b, i], lx[b, i]:rx[b, i]] = True
            out[b, off:off + h * w] = m.reshape(-1).astype(np.float32)
    return out


_NC_CACHE = None


def _get_nc():
    global _NC_CACHE
    if _NC_CACHE is None:
        _NC_CACHE = _build_module()
    return _NC_CACHE


def _run(in_maps, trace=False, trace_cores=None):
    from concourse.bass_utils import run_bass_kernel_spmd

    kwargs = {}
    if trace:
        kwargs.update(trace=True, trace_cores=trace_cores or [0])
    return run_bass_kernel_spmd(_get_nc(), in_maps, core_ids=list(range(N_CORES)),
                                **kwargs)


def _pack_const(inputs):
    """Pack replicated weights/bias: chunk ((lvl*2+oc)*2+kc) at idx*128
    holds w_lvl[oc*128+o_local, kc*128+c_local] transposed."""
    wt_packed = np.zeros((128, N_WT_CHUNKS * 128), np.float32)
    bias_packed = np.zeros((128, N_LEVELS * 2), np.float32)
    for lvl in range(N_LEVELS):
        w = np.asarray(inputs[f"adapt_w{lvl}"], np.float32)
        bvec = np.asarray(inputs[f"adapt_b{lvl}"], np.float32)
        for oc in range(2):
            bias_packed[:, lvl * 2 + oc] = bvec[oc * 128:(oc + 1) * 128]
            for kc in range(2):
                idx = (lvl * 2 + oc) * 2 + kc
                blk = w[oc * 128:(oc + 1) * 128, kc * 128:(kc + 1) * 128]
                wt_packed[:, idx * 128:(idx + 1) * 128] = blk.T
    return wt_packed, bias_packed


def _pack_feat(full, b):
    """[C,*,*] levels of batch item b -> packed [128, 2*MASK_LEN]."""
    cat = np.concatenate(
        [np.asarray(full[l][b], np.float32).reshape(C, HWS[l])
         for l in range(N_LEVELS)], axis=1)          # [256, 21824]
    lohi = np.concatenate([cat[0:128], cat[128:256]], axis=1)  # [128, 2*21824]
    return np.ascontiguousarray(lohi[:, PACK_PERM])


def kernel(_trace=False, _return_results=False, **inputs):
    gt_bboxes = np.asarray(inputs["gt_bboxes"], np.float32)
    masks = _rasterize_masks(gt_bboxes)
    wt_packed, bias_packed = _pack_const(inputs)

    fs_full = [inputs[f"feat_s{l}"] for l in range(N_LEVELS)]
    ft_full = [inputs[f"feat_t{l}"] for l in range(N_LEVELS)]
    in_maps = []
    for b in range(N_CORES):
        in_maps.append({
            "wt": wt_packed, "bias": bias_packed,
            "fs": _pack_feat(fs_full, b),
            "ft": _pack_feat(ft_full, b),
        })

    res = _run(in_maps, trace=_trace)

    s_tot = np.zeros(N_LEVELS, np.float64)
    s_gt = np.zeros(N_LEVELS, np.float64)
    for c in range(N_CORES):
        q = res.results[c]["out_q"].astype(np.float64).reshape(-1)
        for qi, (lvl, col, n) in enumerate(Q_TILES):
            qv = q[qi * TILE_N:qi * TILE_N + n]
            mv = masks[c, MASK_OFF[lvl] + col:MASK_OFF[lvl] + col + n].astype(np.float64)
            s_tot[lvl] += qv.sum()
            s_gt[lvl] += (qv * mv).sum()

    loss = np.float64(0.0)
    for lvl in range(N_LEVELS):
        s_bg = s_tot[lvl] - s_gt[lvl]
        loss += WEIGHT_GT * np.sqrt(s_gt[lvl] + 1e-8) + \
            WEIGHT_BG * np.sqrt(s_bg + 1e-8)

    out = np.array(loss, dtype=np.float32)
    if _return_results:
        return out, res
    return out


# revision 6
# speedup vs baseline: 1.1537x; 1.0086x over previous
"""DeFeat distillation loss on 8 Trainium2 NeuronCores (Bass/Tile).

Data-parallel over the batch dim (B=8 -> 1 batch element per core).
Host packs all 5 pyramid levels into ONE [128, 2*21824] f32 array per
feature tensor (per block: [lo-half cols | hi-half cols]), so the kernel
streams uniform large-line DMAs with a single dma_start per tensor per
block. Per 512-column matmul tile:
  psum = W @ feat_s                      [TensorE fp32r]
  d    = (feat_t - b) - psum   (bf16)    [VectorE fused, frees psum fast]
  dd   = d^2                   (bf16)    [ScalarE Square, segment-wide]
  q    = ones^T @ dd           (psum)    [TensorE bf16: column sums]
  qcat[tile]  = q (bf16 staging row)     [ScalarE copy]
The mask depends only on the column, so the masked sum factors:
  s_gt = sum_n m[n] * q[n],  s_tot = sum_n q[n].
The host rasterizes the masks, takes the per-core q vectors and finishes
both dot products in float64, then applies sqrt + weights.

Block schedule: small blocks first (fast pipeline fill), 2048-wide in
steady state, and a tiny 320-col final block (levels 3+4) so the compute
tail after the last DMA is short.
"""

import os
import sys

for _p in ("/opt/trn_rl_repo", os.path.expanduser("~/.axon_site/_ro/trn_rl_repo")):
    if os.path.isdir(_p) and _p not in sys.path:
        sys.path.insert(0, _p)

import numpy as np

WEIGHT_GT = 0.004
WEIGHT_BG = 0.0002
STRIDES = (8, 16, 32, 64, 128)
SIZES = (128, 64, 32, 16, 8)
HWS = tuple(s * s for s in SIZES)          # (16384, 4096, 1024, 256, 64)
B, C, NBOX = 8, 256, 16
N_CORES = 8
TILE_N = 512                               # matmul free-dim tile
N_LEVELS = 5
MASK_LEN = sum(HWS)                        # 21824
MASK_OFF = tuple(sum(HWS[:i]) for i in range(N_LEVELS))
LEVEL_OFF = MASK_OFF                       # packed column space == level concat

# Widths: narrow head blocks start compute early; 2048 steady-state; the
# last block is levels 3+4 (320 cols) for a short tail.
BLOCK_WIDTHS = (512, 1536) + (2048,) * 9 + (1024, 320)
MAX_BW = max(BLOCK_WIDTHS)
N_WT_CHUNKS = N_LEVELS * 4                 # (lvl, oc, kc) 128x128 chunks


def _layout():
    """Per-block segments/tiles in the packed column space + q-tile table."""
    blocks = []
    q_tiles = []                           # (lvl, level_col, n) per 512-tile
    off = 0
    for w_blk in BLOCK_WIDTHS:
        segs = []                          # (lvl, level_col, block_col, w)
        col, bcol = off, 0
        while col < off + w_blk:
            lvl = max(i for i in range(N_LEVELS) if LEVEL_OFF[i] <= col)
            seg_end = min(off + w_blk, LEVEL_OFF[lvl] + HWS[lvl])
            w = seg_end - col
            segs.append((lvl, col - LEVEL_OFF[lvl], bcol, w))
            col += w
            bcol += w
        mm_tiles = []                      # (block_col, n, lvl, q_index)
        for (lvl, lcol, bcol, w) in segs:
            for j in range(0, w, TILE_N):
                n = min(TILE_N, w - j)
                mm_tiles.append((bcol + j, n, lvl, len(q_tiles)))
                q_tiles.append((lvl, lcol + j, n))
        blocks.append(dict(off=off, w_blk=w_blk, segs=segs, mm_tiles=mm_tiles))
        off += w_blk
    assert off == MASK_LEN
    return blocks, q_tiles


BLOCK_LAYOUT, Q_TILES = _layout()
N_QT = len(Q_TILES)                        # 41
# out DMA split: everything up to the last block's tiles goes out early
N_QT_LAST = len(BLOCK_LAYOUT[-1]["mm_tiles"])
QCOL_A = (N_QT - N_QT_LAST) * TILE_N

# packed column permutation: packed col -> col of [S_lo | S_hi] concat
_perm = []
for _off, _w in zip((b["off"] for b in BLOCK_LAYOUT), BLOCK_WIDTHS):
    _perm.extend(range(_off, _off + _w))
    _perm.extend(range(MASK_LEN + _off, MASK_LEN + _off + _w))
PACK_PERM = np.asarray(_perm, np.int64)


def _build_module():
    import concourse.mybir as mybir
    from concourse import bacc
    from concourse.tile import TileContext

    dt = mybir.dt
    nc = bacc.Bacc("TRN2", target_bir_lowering=False, debug=False,
                   num_devices=N_CORES)

    fs_d = nc.dram_tensor("fs", [128, 2 * MASK_LEN], dt.float32,
                          kind="ExternalInput")
    ft_d = nc.dram_tensor("ft", [128, 2 * MASK_LEN], dt.float32,
                          kind="ExternalInput")
    # weight chunk idx ((lvl*2+oc)*2+kc) at columns idx*128
    wt_d = nc.dram_tensor("wt", [128, N_WT_CHUNKS * 128], dt.float32,
                          kind="ExternalInput")
    bias_d = nc.dram_tensor("bias", [128, N_LEVELS * 2], dt.float32,
                            kind="ExternalInput")
    out_q_d = nc.dram_tensor("out_q", [1, N_QT * TILE_N], dt.bfloat16,
                             kind="ExternalOutput")

    f32r = dt.float32r
    SUB = mybir.AluOpType.subtract
    SQUARE = mybir.ActivationFunctionType.Square

    with TileContext(nc) as tc:
        with (
            tc.tile_pool(name="const", bufs=1) as const_pool,
            tc.tile_pool(name="feat", bufs=3) as feat_pool,
            tc.tile_pool(name="work", bufs=3) as work_pool,
            tc.tile_pool(name="ps", bufs=6, space="PSUM") as psum_pool,
            tc.tile_pool(name="qps", bufs=2, space="PSUM") as qpsum_pool,
        ):
            wt = const_pool.tile([128, N_WT_CHUNKS * 128], f32r)
            bias = const_pool.tile([128, N_LEVELS * 2], dt.float32)
            ones_bf = const_pool.tile([128, 1], dt.bfloat16)
            nc.vector.memset(ones_bf[:], 1.0)
            qcat = const_pool.tile([1, N_QT * TILE_N], dt.bfloat16)
            # zero the partial-tile gap columns (narrow tiles) so the final
            # DMA reads initialized memory
            for _qi, (_lvl, _col, _n) in enumerate(Q_TILES):
                if _n < TILE_N:
                    nc.vector.memset(
                        qcat[:, _qi * TILE_N + _n:(_qi + 1) * TILE_N], 0.0)

            def q_phase(mm_tiles, dd0, dd1):
                # column sums over all 256 channels: q = ones^T @ [dd0;dd1]
                for (bcol, n, lvl, qi) in mm_tiles:
                    qps = qpsum_pool.tile([1, TILE_N], dt.float32, tag="qps")
                    nc.tensor.matmul(qps[:, :n], ones_bf[:],
                                     dd0[:, bcol:bcol + n],
                                     start=True, stop=False)
                    nc.tensor.matmul(qps[:, :n], ones_bf[:],
                                     dd1[:, bcol:bcol + n],
                                     start=False, stop=True)
                    nc.scalar.copy(qcat[:, qi * TILE_N:qi * TILE_N + n],
                                   qps[:, :n])

            pending = None
            for bi, blk in enumerate(BLOCK_LAYOUT):
                w = blk["w_blk"]
                off2 = 2 * blk["off"]
                s_t = feat_pool.tile([128, 2 * MAX_BW], f32r, tag="s")
                t_t = feat_pool.tile([128, 2 * MAX_BW], dt.float32, tag="t")
                # fine-grained lo/hi DMAs keep the matmul->STT pipeline
                # tight (psum banks drain early; PE never bulk-stalls)
                nc.sync.dma_start(out=s_t[:, 0:w],
                                  in_=fs_d[:, off2:off2 + w].bitcast(f32r))
                if bi == 0:
                    # level-0 weights (first 4 chunks) unblock block 0
                    nc.sync.dma_start(out=wt[:, 0:512],
                                      in_=wt_d[:, 0:512].bitcast(f32r))
                nc.sync.dma_start(out=s_t[:, w:2 * w],
                                  in_=fs_d[:, off2 + w:off2 + 2 * w].bitcast(f32r))
                nc.sync.dma_start(out=t_t[:, 0:w], in_=ft_d[:, off2:off2 + w])
                nc.sync.dma_start(out=t_t[:, w:2 * w],
                                  in_=ft_d[:, off2 + w:off2 + 2 * w])
                if bi == 0:
                    # must be emitted before the first STT reads bias
                    nc.sync.dma_start(out=bias[:], in_=bias_d[:])
                elif bi == 1:
                    nc.sync.dma_start(
                        out=wt[:, 512:N_WT_CHUNKS * 128],
                        in_=wt_d[:, 512:N_WT_CHUNKS * 128].bitcast(f32r))

                dd_ocs = []
                for oc in range(2):
                    d_blk = work_pool.tile([128, MAX_BW], dt.bfloat16, tag="d")
                    for (bcol, n, lvl, qi) in blk["mm_tiles"]:
                        widx = (lvl * 2 + oc) * 2
                        ps = psum_pool.tile([128, TILE_N], dt.float32,
                                            tag="ps")
                        nc.tensor.matmul(
                            ps[:, :n],
                            wt[:, widx * 128:(widx + 1) * 128],
                            s_t[:, bcol:bcol + n],
                            start=True, stop=False)
                        nc.tensor.matmul(
                            ps[:, :n],
                            wt[:, (widx + 1) * 128:(widx + 2) * 128],
                            s_t[:, w + bcol:w + bcol + n],
                            start=False, stop=True)
                        # d = (t - b) - psum; frees the psum bank quickly
                        nc.vector.scalar_tensor_tensor(
                            d_blk[:, bcol:bcol + n],
                            t_t[:, oc * w + bcol:oc * w + bcol + n],
                            bias[:, lvl * 2 + oc:lvl * 2 + oc + 1],
                            ps[:, :n],
                            op0=SUB, op1=SUB)
                    dd_blk = work_pool.tile([128, MAX_BW], dt.bfloat16,
                                            tag=f"dd{oc}")
                    for (lvl, lcol, bcol, sw) in blk["segs"]:
                        nc.scalar.activation(
                            dd_blk[:, bcol:bcol + sw],
                            d_blk[:, bcol:bcol + sw], SQUARE)
                    dd_ocs.append(dd_blk)

                # software-pipelined: emit the PREVIOUS block's q phase so
                # the in-order PE stream never waits on this block's squares
                if pending is not None:
                    q_phase(*pending)
                if bi == len(BLOCK_LAYOUT) - 1:
                    # everything but the last block's q tiles is final now
                    nc.sync.dma_start(out=out_q_d[:, 0:QCOL_A],
                                      in_=qcat[:, 0:QCOL_A])
                pending = (blk["mm_tiles"], dd_ocs[0], dd_ocs[1])

            q_phase(*pending)
            nc.sync.dma_start(out=out_q_d[:, QCOL_A:],
                              in_=qcat[:, QCOL_A:])

    nc.compile()
    return nc


def _rasterize_masks(gt_bboxes):
    """Host-side mask rasterization, mirroring reference.gt_mask in fp32.

    Returns [B, MASK_LEN] float32 (per-level masks concatenated)."""
    out = np.zeros((B, MASK_LEN), np.float32)
    for lvl in range(N_LEVELS):
        h = w = SIZES[lvl]
        stride = np.float32(STRIDES[lvl])
        off = MASK_OFF[lvl]
        q = np.floor(gt_bboxes.astype(np.float32) / stride).astype(np.int32)
        lx = np.minimum(q[..., 0], w - 1)
        ly = np.minimum(q[..., 1], h - 1)
        rx = np.minimum(q[..., 2], w - 1)
        ry = np.minimum(q[..., 3], h - 1)
        for b in range(B):
            m = np.zeros((h, w), bool)
            for i in range(gt_bboxes.shape[1]):
                if lx[b, i] == rx[b, i] or ly[b, i] == ry[b, i]:
                    m[ly[b, i], lx[b, i]] = True
                else:
                    m[ly[b, i]:ry[b, i], lx[b, i]:rx[b, i]] = True
            out[b, off:off + h * w] = m.reshape(-1).astype(np.float32)
    return out


_NC_CACHE = None


def _get_nc():
    global _NC_CACHE
    if _NC_CACHE is None:
        _NC_CACHE = _build_module()
    return _NC_CACHE


def _run(in_maps, trace=False, trace_cores=None):
    from concourse.bass_utils import run_bass_kernel_spmd

    kwargs = {}
    if trace:
        kwargs.update(trace=True, trace_cores=trace_cores or [0])
    return run_bass_kernel_spmd(_get_nc(), in_maps, core_ids=list(range(N_CORES)),
                                **kwargs)


def _pack_const(inputs):
    """Pack replicated weights/bias: chunk ((lvl*2+oc)*2+kc) at idx*128
    holds w_lvl[oc*128+o_local, kc*128+c_local] transposed."""
    wt_packed = np.zeros((128, N_WT_CHUNKS * 128), np.float32)
    bias_packed = np.zeros((128, N_LEVELS * 2), np.float32)
    for lvl in range(N_LEVELS):
        w = np.asarray(inputs[f"adapt_w{lvl}"], np.float32)
        bvec = np.asarray(inputs[f"adapt_b{lvl}"], np.float32)
        for oc in range(2):
            bias_packed[:, lvl * 2 + oc] = bvec[oc * 128:(oc + 1) * 128]
            for kc in range(2):
                idx = (lvl * 2 + oc) * 2 + kc
                blk = w[oc * 128:(oc + 1) * 128, kc * 128:(kc + 1) * 128]
                wt_packed[:, idx * 128:(idx + 1) * 128] = blk.T
    return wt_packed, bias_packed


def _pack_feat(full, b):
    """[C,*,*] levels of batch item b -> packed [128, 2*MASK_LEN]."""
    cat = np.concatenate(
        [np.asarray(full[l][b], np.float32).reshape(C, HWS[l])
         for l in range(N_LEVELS)], axis=1)          # [256, 21824]
    lohi = np.concatenate([cat[0:128], cat[128:256]], axis=1)  # [128, 2*21824]
    return np.ascontiguousarray(lohi[:, PACK_PERM])


def kernel(_trace=False, _return_results=False, **inputs):
    gt_bboxes = np.asarray(inputs["gt_bboxes"], np.float32)
    masks = _rasterize_masks(gt_bboxes)
    wt_packed, bias_packed = _pack_const(inputs)

    fs_full = [inputs[f"feat_s{l}"] for l in range(N_LEVELS)]
    ft_full = [inputs[f"feat_t{l}"] for l in range(N_LEVELS)]
    in_maps = []
    for b in range(N_CORES):
        in_maps.append({
            "wt": wt_packed, "bias": bias_packed,
            "fs": _pack_feat(fs_full, b),
            "ft": _pack_feat(ft_full, b),
        })

    res = _run(in_maps, trace=_trace)

    s_tot = np.zeros(N_LEVELS, np.float64)
    s_gt = np.zeros(N_LEVELS, np.float64)
    for c in range(N_CORES):
        q = res.results[c]["out_q"].astype(np.float64).reshape(-1)
        for qi, (lvl, col, n) in enumerate(Q_TILES):
            qv = q[qi * TILE_N:qi * TILE_N + n]
            mv = masks[c, MASK_OFF[lvl] + col:MASK_OFF[lvl] + col + n].astype(np.float64)
            s_tot[lvl] += qv.sum()
            s_gt[lvl] += (qv * mv).sum()

    loss = np.float64(0.0)
    for lvl in range(N_LEVELS):
        s_bg = s_tot[lvl] - s_gt[lvl]
        loss += WEIGHT_GT * np.sqrt(s_gt[lvl] + 1e-8) + \
            WEIGHT_BG * np.sqrt(s_bg + 1e-8)

    out = np.array(loss, dtype=np.float32)
    if _return_results:
        return out, res
    return out
